# revision 9
# baseline (speedup 1.0000x reference)
"""Trainium2 Bass kernel for the BioRNN problem.

Math (per batch element b):
    Wih_m = W_ih * mask_ih            [H, I]
    Whh_m = W_hh * mask_hh            [H, H]
    xp[t] = Wih_m @ x[t] + b_ih + b_hh
    h[t]  = tanh(xp[t] + Whh_m @ h[t-1])
    out[t] = W_fc @ h[t] + b_fc

Strategy: data-parallel over batch (B=64 -> 8 per NeuronCore), weights
replicated, the T=2048 time scan runs locally per core with no
cross-core communication.

v2 design (measured on HW: the v1 kernel spent ~460ns/step of PE idle
waiting on the matmul-drain + tanh round-trip, with the 18 LDW+MM pairs
issuing at ~26.5ns/pair):
  - The 8 local batch elements split into TWO independent groups of 4.
    The groups' recurrences interleave: group B's matmul burst executes
    while group A's tanh is in flight, hiding the ACT round-trip.
  - x is pre-transposed on the HOST to [i, (t, b)] fp16; the readout is
    written transposed [o, t] per b and un-transposed on the host. This
    removes all PE transposes and DVE casts (and all fp32 matmuls, which
    block FWL fast weight loads).
  - Recurrence weights are fp8e3m4, prescaled by WSCALE (undone via the
    tanh's free scale); moving operands stay fp16, PSUM fp32.
"""

import numpy as np

import concourse.bacc as bacc
import concourse.mybir as mybir
import concourse.tile as tile
from concourse.bass import ds, ts
from concourse.masks import make_identity
from concourse.bass_utils import run_bass_kernel_spmd

F32 = mybir.dt.float32
F16 = mybir.dt.float16
F8 = mybir.dt.float8e3     # e3m4: 4 mantissa bits, max 15.5
AFT = mybir.ActivationFunctionType

B, T, I, H, O = 64, 2048, 128, 512, 128
NCORES = 8
BL = B // NCORES            # 8 batch elements per core
KJ = H // 128               # 4 hidden chunks
NG = 2                      # interleaved recurrence groups
GB = BL // NG               # batch per group
WSCALE = 32.0               # Whh prescale for fp8e3m4; undone in tanh

_cache = {}


def build_rnn(t_total=T, tc=512, static_rhs=False, no_act=False,
              dyn_repeat=False, w8=True, psum_z=4, split_act=False):
    """Build (and bacc-compile) the per-core Bass program.

    static_rhs/no_act are timing-diagnostic switches (wrong results).
    dyn_repeat wraps the computation in a hardware loop driven by an
    "nrep" input for slope-based device timing.
    """
    assert t_total % tc == 0 and tc % 128 == 0
    nt = t_total // tc       # number of time chunks
    nblk = tc * BL // 512    # 512-col blocks per chunk

    nc = bacc.Bacc("TRN2", target_bir_lowering=False, debug=False,
                   num_devices=NCORES)

    FW = F8 if w8 else F16   # stationary dtype for the recurrence weights
    xT_d = nc.dram_tensor("xT", [128, t_total * BL], F16,
                          kind="ExternalInput")                      # [i,(t,b)]
    whhT_d = nc.dram_tensor("whhT", [H, H], FW, kind="ExternalInput")  # [k, j]
    wihT_d = nc.dram_tensor("wihT", [I, H], F16, kind="ExternalInput")  # [i, j]
    wfcT_d = nc.dram_tensor("wfcT", [H, O], F16, kind="ExternalInput")  # [k, o]
    bh_d = nc.dram_tensor("bh", [H], F32, kind="ExternalInput")     # (bih+bhh)*s
    bfc_d = nc.dram_tensor("bfc", [O], F32, kind="ExternalInput")
    h0_d = nc.dram_tensor("h0r", [128, KJ * BL], F16, kind="ExternalInput")
    nrep_d = (nc.dram_tensor("nrep", [1, 1], mybir.dt.int32,
                             kind="ExternalInput") if dyn_repeat else None)
    out_d = nc.dram_tensor("out", [BL, O, t_total], F32, kind="ExternalOutput")

    with tile.TileContext(nc) as tc_ctx:
        with (
            tc_ctx.tile_pool(name="const", bufs=1) as cpool,
            tc_ctx.tile_pool(name="xT", bufs=2) as xT_pool,
            tc_ctx.tile_pool(name="xp", bufs=2) as xp_pool,
            tc_ctx.tile_pool(name="hs", bufs=2) as hs_pool,
            tc_ctx.tile_pool(name="outT", bufs=2) as outT_pool,
            tc_ctx.tile_pool(name="ppz", bufs=psum_z, space="PSUM") as ppz_pool,
            tc_ctx.tile_pool(name="pbig", bufs=8 - psum_z,
                             space="PSUM") as pbig_pool,
        ):
            # ---- constants ----
            ident = cpool.tile([128, 128], FW)
            make_identity(nc, ident[:])

            wT = cpool.tile([128, KJ * H], FW)       # [k-part, (kc, j)]
            nc.sync.dma_start(wT[:].rearrange("p (c j) -> p c j", c=KJ),
                              whhT_d[:].rearrange("(c p) j -> p c j", p=128))
            wih = cpool.tile([128, H], F16)          # [i, j]
            nc.sync.dma_start(wih[:], wihT_d[:])
            wfc = cpool.tile([128, KJ * O], F16)     # [k-part, (kc, o)]
            nc.sync.dma_start(wfc[:].rearrange("p (c o) -> p c o", c=KJ),
                              wfcT_d[:].rearrange("(c p) o -> p c o", p=128))
            bh = cpool.tile([128, KJ], F32)
            nc.sync.dma_start(bh[:], bh_d[:].rearrange("(c p) -> p c", p=128))
            bfc = cpool.tile([128, 1], F32)
            nc.sync.dma_start(bfc[:], bfc_d[:].rearrange("(p o) -> p o", o=1))
            h0sb = cpool.tile([128, KJ * BL], F16)
            nc.sync.dma_start(h0sb[:], h0_d[:])

            def stage1_unit(c, xT):
                """one DMA: the chunk's pre-transposed x [i, (t, b)] fp16"""
                nc.sync.dma_start(xT[:], xT_d[:, ds(c * tc * BL, tc * BL)])

            def stage2_unit(xT, xp, jc, blk):
                """one block of xp = WihT.T @ xT + bias, layout (t, jc, b)"""
                xp_r = xp[:].rearrange("p (t j b) -> p t j b", j=KJ, b=BL)
                pp = pbig_pool.tile([128, 512], F32, tag="big")
                nc.tensor.matmul(pp[:], wih[:, ts(jc, 128)],
                                 xT[:, ds(blk * 512, 512)],
                                 start=True, stop=True)
                nc.vector.tensor_scalar_add(
                    xp_r[:, ds(blk * 64, 64), jc, :],
                    pp[:].rearrange("p (t b) -> p t b", b=BL),
                    bh[:, ds(jc, 1)])

            def stage12_units(c, xT, xp):
                yield lambda: stage1_unit(c, xT)
                for jc in range(KJ):
                    for blk in range(nblk):
                        yield lambda jc=jc, blk=blk: stage2_unit(xT, xp, jc, blk)

            def stage3_recur(c, xp, hs_prev_r, pending=()):
                xp_r = xp[:].rearrange("p (t j b) -> p t j b", j=KJ, b=BL)
                hs = hs_pool.tile([128, KJ * tc * BL], F16)
                hs_r = hs[:].rearrange("p (k t b) -> p k t b", k=KJ, b=BL)

                def rhs_for(t, kc, g):
                    if static_rhs:
                        return h0sb[:, ds(kc * BL + g * GB, GB)]
                    if t > 0:
                        return hs_r[:, kc, t - 1, ds(g * GB, GB)]
                    if c > 0:
                        return hs_prev_r[:, kc, tc - 1, ds(g * GB, GB)]
                    return h0sb[:, ds(kc * BL + g * GB, GB)]

                def gstep(t, g):
                    # one group's step: inject xp, 16 recurrence matmuls
                    # (kc-outer), one tanh over the whole [j, gb] block
                    pz = ppz_pool.tile([128, KJ * GB], F32, tag="pz")
                    pz_r = pz[:].rearrange("p (j b) -> p j b", b=GB)
                    nc.tensor.matmul(pz[:], ident[:],
                                     xp_r[:, t, :, ds(g * GB, GB)],
                                     start=True, stop=False,
                                     skip_group_check=True)
                    for kc in range(KJ):
                        for jc in range(KJ):
                            nc.tensor.matmul(
                                pz_r[:, jc, :],
                                wT[:, ds(kc * H + jc * 128, 128)],
                                rhs_for(t, kc, g), start=False,
                                stop=(kc == KJ - 1),
                                skip_group_check=True)
                    if not no_act:
                        sc = 1.0 / WSCALE if w8 else 1.0
                        if split_act:
                            for s in range(2):
                                nc.scalar.activation(
                                    hs_r[:, ds(2 * s, 2), t, ds(g * GB, GB)],
                                    pz_r[:, ds(2 * s, 2), :], AFT.Tanh,
                                    scale=sc)
                        else:
                            nc.scalar.activation(
                                hs_r[:, :, t, ds(g * GB, GB)],
                                pz_r[:, :, :], AFT.Tanh, scale=sc)

                n_pend = len(pending)
                emitted = 0
                nu = tc * NG
                u = 0
                for t in range(tc):
                    for g in range(NG):
                        gstep(t, g)
                        u += 1
                        # spread boundary work (prev readout, next load/
                        # xproj) between group bursts, where the PE's
                        # dependency slack lives
                        if (u * n_pend) // nu > emitted:
                            pending[emitted]()
                            emitted += 1
                while emitted < n_pend:
                    pending[emitted]()
                    emitted += 1
                if no_act:
                    nc.vector.memset(hs[:], 0.0)
                return hs, hs_r

            def stage4_unit(hs_r, outT, blk):
                """one readout block: outT[o, (b, t)] = WfcT.T @ hs + b_fc"""
                outT_tb = outT[:].rearrange("p (b t) -> p t b", b=BL)
                po = pbig_pool.tile([128, 512], F32, tag="big")
                for kc in range(KJ):
                    nc.tensor.matmul(
                        po[:], wfc[:, ts(kc, 128)],
                        hs_r[:, kc, ds(blk * 64, 64), :],
                        start=(kc == 0), stop=(kc == KJ - 1))
                nc.vector.tensor_scalar_add(
                    outT_tb[:, ds(blk * 64, 64), :],
                    po[:].rearrange("p (t b) -> p t b", b=BL),
                    bfc[:, 0:1])

            def stage5_unit(c, outT, b):
                """DMA one batch row [o, t] of the chunk out (transposed)"""
                nc.sync.dma_start(out_d[b, :, ds(c * tc, tc)],
                                  outT[:, ds(b * tc, tc)])

            def stage45_units(c, hs_r):
                outT = outT_pool.tile([128, tc * BL], F32)
                for blk in range(nblk):
                    yield lambda blk=blk: stage4_unit(hs_r, outT, blk)
                for b in range(BL):
                    yield lambda b=b: stage5_unit(c, outT, b)

            def emit_all():
                # chunk 0 prologue
                xT = xT_pool.tile([128, tc * BL], F16, tag="xT")
                xp = xp_pool.tile([128, tc * KJ * BL], F16, tag="xp")
                for u in stage12_units(0, xT, xp):
                    u()
                hs_prev_r = None
                for c in range(nt):
                    # work to interleave into this chunk's recurrence:
                    # previous chunk's readout + next chunk's load/xproj
                    pending = []
                    if hs_prev_r is not None:
                        pending.extend(stage45_units(c - 1, hs_prev_r))
                    if c + 1 < nt:
                        xT_n = xT_pool.tile([128, tc * BL], F16, tag="xT")
                        xp_n = xp_pool.tile([128, tc * KJ * BL], F16, tag="xp")
                        pending.extend(stage12_units(c + 1, xT_n, xp_n))
                    else:
                        xT_n = xp_n = None
                    hs, hs_r = stage3_recur(c, xp, hs_prev_r, pending)
                    hs_prev_r = hs_r
                    xT, xp = xT_n, xp_n
                # last chunk epilogue
                for u in stage45_units(nt - 1, hs_prev_r):
                    u()

            if dyn_repeat:
                nrep_sb = cpool.tile([1, 1], mybir.dt.int32)
                nc.sync.dma_start(nrep_sb[:], nrep_d[:])
                rep_val = nc.values_load(nrep_sb[0:1, 0:1], min_val=0,
                                         max_val=64,
                                         skip_runtime_bounds_check=True)
                with tc_ctx.For_i(0, rep_val, 1):
                    emit_all()
            else:
                emit_all()

    nc.compile()
    return nc


def _prep_in_maps(x, h0, W_ih, b_ih, W_hh, b_hh, mask_ih, mask_hh, W_fc, b_fc,
                  t_total=T, w8=True):
    # With w8, the whole pre-tanh accumulation (Whh@h and xp alike) is
    # scaled by WSCALE so Whh fits fp8e3m4 well; tanh's scale undoes it.
    s = WSCALE if w8 else 1.0
    if w8:
        import ml_dtypes
        whhT = np.ascontiguousarray(
            (np.asarray(W_hh) * np.asarray(mask_hh)).T * s).astype(
                ml_dtypes.float8_e3m4)
    else:
        whhT = np.ascontiguousarray(
            (np.asarray(W_hh) * np.asarray(mask_hh)).T).astype(np.float16)
    wihT = np.ascontiguousarray(
        (np.asarray(W_ih) * np.asarray(mask_ih)).T * s).astype(np.float16)
    wfcT = np.ascontiguousarray(np.asarray(W_fc).T).astype(np.float16)
    bh = ((np.asarray(b_ih) + np.asarray(b_hh)) * s).astype(np.float32)
    bfc = np.asarray(b_fc).astype(np.float32)
    x = np.asarray(x)
    h0 = np.asarray(h0)
    in_maps = []
    for core in range(NCORES):
        bs = core * BL
        h0s = h0[0, bs:bs + BL, :].astype(np.float16)          # [BL, H]
        h0r = np.ascontiguousarray(
            h0s.T.reshape(KJ, 128, BL).transpose(1, 0, 2).reshape(128, KJ * BL))
        # [BL, T, I] -> [I, T, BL] fp16, flattened [128, T*BL]
        xT = np.ascontiguousarray(
            x[bs:bs + BL, :t_total, :].transpose(2, 1, 0)).astype(
                np.float16).reshape(128, t_total * BL)
        in_maps.append({
            "xT": xT,
            "whhT": whhT, "wihT": wihT, "wfcT": wfcT,
            "bh": bh, "bfc": bfc, "h0r": h0r,
        })
    return in_maps


def kernel(x, h0, W_ih, b_ih, W_hh, b_hh, mask_ih, mask_hh, W_fc, b_fc):
    if "nc" not in _cache:
        _cache["nc"] = build_rnn()
    nc = _cache["nc"]
    in_maps = _prep_in_maps(x, h0, W_ih, b_ih, W_hh, b_hh,
                            mask_ih, mask_hh, W_fc, b_fc)
    res = run_bass_kernel_spmd(nc, in_maps, list(range(NCORES)))
    # device output is [BL, O, T]; un-transpose to [BL, T, O]
    return np.concatenate(
        [res.results[c]["out"].transpose(0, 2, 1) for c in range(NCORES)],
        axis=0).astype(np.float32)


# revision 15
# speedup vs baseline: 1.0001x; 1.0001x over previous
"""Trainium2 Bass kernel for the BioRNN problem.

Math (per batch element b):
    Wih_m = W_ih * mask_ih            [H, I]
    Whh_m = W_hh * mask_hh            [H, H]
    xp[t] = Wih_m @ x[t] + b_ih + b_hh
    h[t]  = tanh(xp[t] + Whh_m @ h[t-1])
    out[t] = W_fc @ h[t] + b_fc

Strategy: data-parallel over batch (B=64 -> 8 per NeuronCore), weights
replicated, the T=2048 time scan runs locally per core with no
cross-core communication.

v2 design (measured on HW: the v1 kernel spent ~460ns/step of PE idle
waiting on the matmul-drain + tanh round-trip, with the 18 LDW+MM pairs
issuing at ~26.5ns/pair):
  - The 8 local batch elements split into TWO independent groups of 4.
    The groups' recurrences interleave: group B's matmul burst executes
    while group A's tanh is in flight, hiding the ACT round-trip.
  - x is pre-transposed on the HOST to [i, (t, b)] fp16; the readout is
    written transposed [o, t] per b and un-transposed on the host. This
    removes all PE transposes and DVE casts (and all fp32 matmuls, which
    block FWL fast weight loads).
  - Recurrence weights are fp8e3m4, prescaled by WSCALE (undone via the
    tanh's free scale); moving operands stay fp16, PSUM fp32.
"""

import numpy as np

import concourse.bacc as bacc
import concourse.mybir as mybir
import concourse.tile as tile
from concourse.bass import ds, ts
from concourse.masks import make_identity
from concourse.bass_utils import run_bass_kernel_spmd

F32 = mybir.dt.float32
F16 = mybir.dt.float16
BF16 = mybir.dt.bfloat16
F8 = mybir.dt.float8e3     # e3m4: 4 mantissa bits, max 15.5
AFT = mybir.ActivationFunctionType

B, T, I, H, O = 64, 2048, 128, 512, 128
NCORES = 8
BL = B // NCORES            # 8 batch elements per core
KJ = H // 128               # 4 hidden chunks
NG = 2                      # interleaved recurrence groups
GB = BL // NG               # batch per group
WSCALE = 32.0               # Whh prescale for fp8e3m4; undone in tanh

_cache = {}


def build_rnn(t_total=T, tc=512, static_rhs=False, no_act=False,
              dyn_repeat=False, wdtype="f8", psum_z=4, split_act=False):
    """Build (and bacc-compile) the per-core Bass program.

    static_rhs/no_act are timing-diagnostic switches (wrong results).
    dyn_repeat wraps the computation in a hardware loop driven by an
    "nrep" input for slope-based device timing.
    """
    assert t_total % tc == 0 and tc % 128 == 0
    nt = t_total // tc       # number of time chunks
    nblk = tc * BL // 512    # 512-col blocks per chunk

    nc = bacc.Bacc("TRN2", target_bir_lowering=False, debug=False,
                   num_devices=NCORES)

    w8 = wdtype == "f8"
    FW = {"f8": F8, "f16": F16, "bf16": BF16}[wdtype]  # recurrence stationary
    xT_d = nc.dram_tensor("xT", [128, t_total * BL], F16,
                          kind="ExternalInput")                      # [i,(t,b)]
    whhT_d = nc.dram_tensor("whhT", [H, H], FW, kind="ExternalInput")  # [k, j]
    wihT_d = nc.dram_tensor("wihT", [I, H], F16, kind="ExternalInput")  # [i, j]
    wfcT_d = nc.dram_tensor("wfcT", [H, O], F16, kind="ExternalInput")  # [k, o]
    bh_d = nc.dram_tensor("bh", [H], F32, kind="ExternalInput")     # (bih+bhh)*s
    bfc_d = nc.dram_tensor("bfc", [O], F32, kind="ExternalInput")
    h0_d = nc.dram_tensor("h0r", [128, KJ * BL], F16, kind="ExternalInput")
    nrep_d = (nc.dram_tensor("nrep", [1, 1], mybir.dt.int32,
                             kind="ExternalInput") if dyn_repeat else None)
    out_d = nc.dram_tensor("out", [BL, O, t_total], F32, kind="ExternalOutput")

    with tile.TileContext(nc) as tc_ctx:
        with (
            tc_ctx.tile_pool(name="const", bufs=1) as cpool,
            tc_ctx.tile_pool(name="xT", bufs=2) as xT_pool,
            tc_ctx.tile_pool(name="xp", bufs=2) as xp_pool,
            tc_ctx.tile_pool(name="hs", bufs=2) as hs_pool,
            tc_ctx.tile_pool(name="outT", bufs=2) as outT_pool,
            tc_ctx.tile_pool(name="ppza", bufs=psum_z // 2,
                             space="PSUM") as ppza_pool,
            tc_ctx.tile_pool(name="ppzb", bufs=psum_z // 2,
                             space="PSUM") as ppzb_pool,
            tc_ctx.tile_pool(name="pbig", bufs=8 - psum_z,
                             space="PSUM") as pbig_pool,
        ):
            # ---- constants ----
            ident = cpool.tile([128, 128], FW)
            make_identity(nc, ident[:])

            wT = cpool.tile([128, KJ * H], FW)       # [k-part, (kc, j)]
            nc.sync.dma_start(wT[:].rearrange("p (c j) -> p c j", c=KJ),
                              whhT_d[:].rearrange("(c p) j -> p c j", p=128))
            wih = cpool.tile([128, H], F16)          # [i, j]
            nc.sync.dma_start(wih[:], wihT_d[:])
            wfc = cpool.tile([128, KJ * O], F16)     # [k-part, (kc, o)]
            nc.sync.dma_start(wfc[:].rearrange("p (c o) -> p c o", c=KJ),
                              wfcT_d[:].rearrange("(c p) o -> p c o", p=128))
            bh = cpool.tile([128, KJ], F32)
            nc.sync.dma_start(bh[:], bh_d[:].rearrange("(c p) -> p c", p=128))
            bfc = cpool.tile([128, 1], F32)
            nc.sync.dma_start(bfc[:], bfc_d[:].rearrange("(p o) -> p o", o=1))
            h0sb = cpool.tile([128, KJ * BL], F16)
            nc.sync.dma_start(h0sb[:], h0_d[:])

            def stage1_unit(c, xT):
                """one DMA: the chunk's pre-transposed x [i, (t, b)] fp16"""
                nc.sync.dma_start(xT[:], xT_d[:, ds(c * tc * BL, tc * BL)])

            def stage2_unit(xT, xp, jc, blk):
                """one block of xp = WihT.T @ xT + bias, layout (t, jc, b)"""
                xp_r = xp[:].rearrange("p (t j b) -> p t j b", j=KJ, b=BL)
                pp = pbig_pool.tile([128, 512], F32, tag="big")
                nc.tensor.matmul(pp[:], wih[:, ts(jc, 128)],
                                 xT[:, ds(blk * 512, 512)],
                                 start=True, stop=True)
                nc.vector.tensor_scalar_add(
                    xp_r[:, ds(blk * 64, 64), jc, :],
                    pp[:].rearrange("p (t b) -> p t b", b=BL),
                    bh[:, ds(jc, 1)])

            def stage12_units(c, xT, xp):
                yield lambda: stage1_unit(c, xT)
                for jc in range(KJ):
                    for blk in range(nblk):
                        yield lambda jc=jc, blk=blk: stage2_unit(xT, xp, jc, blk)

            def stage3_recur(c, xp, hs_prev_r, pending=()):
                xp_r = xp[:].rearrange("p (t j b) -> p t j b", j=KJ, b=BL)
                hs = hs_pool.tile([128, KJ * tc * BL], F16)
                hs_r = hs[:].rearrange("p (k t b) -> p k t b", k=KJ, b=BL)

                def rhs_for(t, kc, g):
                    if static_rhs:
                        return h0sb[:, ds(kc * BL + g * GB, GB)]
                    if t > 0:
                        return hs_r[:, kc, t - 1, ds(g * GB, GB)]
                    if c > 0:
                        return hs_prev_r[:, kc, tc - 1, ds(g * GB, GB)]
                    return h0sb[:, ds(kc * BL + g * GB, GB)]

                def gstep(t, g):
                    # one group's step: inject xp, 16 recurrence matmuls
                    # (kc-outer), one tanh over the whole [j, gb] block
                    pool = ppza_pool if g == 0 else ppzb_pool
                    pz = pool.tile([128, KJ * GB], F32, tag="pz")
                    pz_r = pz[:].rearrange("p (j b) -> p j b", b=GB)
                    nc.tensor.matmul(pz[:], ident[:],
                                     xp_r[:, t, :, ds(g * GB, GB)],
                                     start=True, stop=False,
                                     skip_group_check=True)
                    for kc in range(KJ):
                        for jc in range(KJ):
                            nc.tensor.matmul(
                                pz_r[:, jc, :],
                                wT[:, ds(kc * H + jc * 128, 128)],
                                rhs_for(t, kc, g), start=False,
                                stop=(kc == KJ - 1),
                                skip_group_check=True)
                    if not no_act:
                        sc = 1.0 / WSCALE if w8 else 1.0
                        if split_act:
                            for s in range(2):
                                nc.scalar.activation(
                                    hs_r[:, ds(2 * s, 2), t, ds(g * GB, GB)],
                                    pz_r[:, ds(2 * s, 2), :], AFT.Tanh,
                                    scale=sc)
                        else:
                            nc.scalar.activation(
                                hs_r[:, :, t, ds(g * GB, GB)],
                                pz_r[:, :, :], AFT.Tanh, scale=sc)

                n_pend = len(pending)
                emitted = 0
                nu = tc * NG
                u = 0
                for t in range(tc):
                    for g in range(NG):
                        gstep(t, g)
                        u += 1
                        # spread boundary work (prev readout, next load/
                        # xproj) between group bursts, where the PE's
                        # dependency slack lives
                        if (u * n_pend) // nu > emitted:
                            pending[emitted]()
                            emitted += 1
                while emitted < n_pend:
                    pending[emitted]()
                    emitted += 1
                if no_act:
                    nc.vector.memset(hs[:], 0.0)
                return hs, hs_r

            def stage4_unit(hs_r, outT, blk):
                """one readout block: outT[o, (b, t)] = WfcT.T @ hs + b_fc"""
                outT_tb = outT[:].rearrange("p (b t) -> p t b", b=BL)
                po = pbig_pool.tile([128, 512], F32, tag="big")
                for kc in range(KJ):
                    nc.tensor.matmul(
                        po[:], wfc[:, ts(kc, 128)],
                        hs_r[:, kc, ds(blk * 64, 64), :],
                        start=(kc == 0), stop=(kc == KJ - 1))
                nc.vector.tensor_scalar_add(
                    outT_tb[:, ds(blk * 64, 64), :],
                    po[:].rearrange("p (t b) -> p t b", b=BL),
                    bfc[:, 0:1])

            def stage5_unit(c, outT, b):
                """DMA one batch row [o, t] of the chunk out (transposed)"""
                nc.sync.dma_start(out_d[b, :, ds(c * tc, tc)],
                                  outT[:, ds(b * tc, tc)])

            def stage45_units(c, hs_r):
                outT = outT_pool.tile([128, tc * BL], F32)
                for blk in range(nblk):
                    yield lambda blk=blk: stage4_unit(hs_r, outT, blk)
                for b in range(BL):
                    yield lambda b=b: stage5_unit(c, outT, b)

            def emit_all():
                # chunk 0 prologue
                xT = xT_pool.tile([128, tc * BL], F16, tag="xT")
                xp = xp_pool.tile([128, tc * KJ * BL], F16, tag="xp")
                for u in stage12_units(0, xT, xp):
                    u()
                hs_prev_r = None
                for c in range(nt):
                    # work to interleave into this chunk's recurrence:
                    # previous chunk's readout + next chunk's load/xproj
                    pending = []
                    if hs_prev_r is not None:
                        pending.extend(stage45_units(c - 1, hs_prev_r))
                    if c + 1 < nt:
                        xT_n = xT_pool.tile([128, tc * BL], F16, tag="xT")
                        xp_n = xp_pool.tile([128, tc * KJ * BL], F16, tag="xp")
                        pending.extend(stage12_units(c + 1, xT_n, xp_n))
                    else:
                        xT_n = xp_n = None
                    hs, hs_r = stage3_recur(c, xp, hs_prev_r, pending)
                    hs_prev_r = hs_r
                    xT, xp = xT_n, xp_n
                # last chunk epilogue
                for u in stage45_units(nt - 1, hs_prev_r):
                    u()

            if dyn_repeat:
                nrep_sb = cpool.tile([1, 1], mybir.dt.int32)
                nc.sync.dma_start(nrep_sb[:], nrep_d[:])
                rep_val = nc.values_load(nrep_sb[0:1, 0:1], min_val=0,
                                         max_val=64,
                                         skip_runtime_bounds_check=True)
                with tc_ctx.For_i(0, rep_val, 1):
                    emit_all()
            else:
                emit_all()

    nc.compile()
    return nc


def _prep_in_maps(x, h0, W_ih, b_ih, W_hh, b_hh, mask_ih, mask_hh, W_fc, b_fc,
                  t_total=T, wdtype="f8"):
    # With fp8 weights, the whole pre-tanh accumulation (Whh@h and xp
    # alike) is scaled by WSCALE so Whh fits fp8e3m4 well; tanh's scale
    # undoes it.
    w8 = wdtype == "f8"
    s = WSCALE if w8 else 1.0
    import ml_dtypes
    wnp = {"f8": ml_dtypes.float8_e3m4, "f16": np.float16,
           "bf16": ml_dtypes.bfloat16}[wdtype]
    whhT = np.ascontiguousarray(
        (np.asarray(W_hh) * np.asarray(mask_hh)).T * s).astype(wnp)
    wihT = np.ascontiguousarray(
        (np.asarray(W_ih) * np.asarray(mask_ih)).T * s).astype(np.float16)
    wfcT = np.ascontiguousarray(np.asarray(W_fc).T).astype(np.float16)
    bh = ((np.asarray(b_ih) + np.asarray(b_hh)) * s).astype(np.float32)
    bfc = np.asarray(b_fc).astype(np.float32)
    x = np.asarray(x)
    h0 = np.asarray(h0)
    in_maps = []
    for core in range(NCORES):
        bs = core * BL
        h0s = h0[0, bs:bs + BL, :].astype(np.float16)          # [BL, H]
        h0r = np.ascontiguousarray(
            h0s.T.reshape(KJ, 128, BL).transpose(1, 0, 2).reshape(128, KJ * BL))
        # [BL, T, I] -> [I, T, BL] fp16, flattened [128, T*BL]
        xT = np.ascontiguousarray(
            x[bs:bs + BL, :t_total, :].transpose(2, 1, 0)).astype(
                np.float16).reshape(128, t_total * BL)
        in_maps.append({
            "xT": xT,
            "whhT": whhT, "wihT": wihT, "wfcT": wfcT,
            "bh": bh, "bfc": bfc, "h0r": h0r,
        })
    return in_maps


def kernel(x, h0, W_ih, b_ih, W_hh, b_hh, mask_ih, mask_hh, W_fc, b_fc):
    if "nc" not in _cache:
        _cache["nc"] = build_rnn()
    nc = _cache["nc"]
    in_maps = _prep_in_maps(x, h0, W_ih, b_ih, W_hh, b_hh,
                            mask_ih, mask_hh, W_fc, b_fc)
    res = run_bass_kernel_spmd(nc, in_maps, list(range(NCORES)))
    # device output is [BL, O, T]; un-transpose to [BL, T, O]
    return np.concatenate(
        [res.results[c]["out"].transpose(0, 2, 1) for c in range(NCORES)],
        axis=0).astype(np.float32)


# revision 16
# speedup vs baseline: 1.1976x; 1.1975x over previous
"""Trainium2 Bass kernel for the BioRNN problem.

Math (per batch element b):
    Wih_m = W_ih * mask_ih            [H, I]
    Whh_m = W_hh * mask_hh            [H, H]
    xp[t] = Wih_m @ x[t] + b_ih + b_hh
    h[t]  = tanh(xp[t] + Whh_m @ h[t-1])
    out[t] = W_fc @ h[t] + b_fc

Strategy: data-parallel over batch (B=64 -> 8 per NeuronCore), weights
replicated, the T=2048 time scan runs locally per core with no
cross-core communication.

v2 design (measured on HW: the v1 kernel spent ~460ns/step of PE idle
waiting on the matmul-drain + tanh round-trip, with the 18 LDW+MM pairs
issuing at ~26.5ns/pair):
  - The 8 local batch elements split into TWO independent groups of 4.
    The groups' recurrences interleave: group B's matmul burst executes
    while group A's tanh is in flight, hiding the ACT round-trip.
  - x is pre-transposed on the HOST to [i, (t, b)] fp16; the readout is
    written transposed [o, t] per b and un-transposed on the host. This
    removes all PE transposes and DVE casts (and all fp32 matmuls, which
    block FWL fast weight loads).
  - Recurrence weights are fp8e3m4, prescaled by WSCALE (undone via the
    tanh's free scale); moving operands stay fp16, PSUM fp32.
"""

import numpy as np

import concourse.bacc as bacc
import concourse.mybir as mybir
import concourse.tile as tile
from concourse.bass import ds, ts
from concourse.masks import make_identity
from concourse.bass_utils import run_bass_kernel_spmd

F32 = mybir.dt.float32
F16 = mybir.dt.float16
BF16 = mybir.dt.bfloat16
F8 = mybir.dt.float8e3     # e3m4: 4 mantissa bits, max 15.5
AFT = mybir.ActivationFunctionType

B, T, I, H, O = 64, 2048, 128, 512, 128
NCORES = 8
BL = B // NCORES            # 8 batch elements per core
KJ = H // 128               # 4 hidden chunks
NG = 2                      # interleaved recurrence groups
GB = BL // NG               # batch per group
WSCALE = 32.0               # Whh prescale for fp8e3m4; undone in tanh

_cache = {}


def build_rnn(t_total=T, tc=512, static_rhs=False, no_act=False,
              dyn_repeat=False, wdtype="f8", psum_z=4, split_act=False):
    """Build (and bacc-compile) the per-core Bass program.

    static_rhs/no_act are timing-diagnostic switches (wrong results).
    dyn_repeat wraps the computation in a hardware loop driven by an
    "nrep" input for slope-based device timing.
    """
    assert t_total % tc == 0 and tc % 128 == 0
    nt = t_total // tc       # number of time chunks
    nblk = tc * BL // 512    # 512-col blocks per chunk

    nc = bacc.Bacc("TRN2", target_bir_lowering=False, debug=False,
                   num_devices=NCORES)

    w8 = wdtype == "f8"
    FW = {"f8": F8, "f16": F16, "bf16": BF16}[wdtype]  # recurrence stationary
    xT_d = nc.dram_tensor("xT", [128, t_total * BL], F16,
                          kind="ExternalInput")                      # [i,(t,b)]
    whhT_d = nc.dram_tensor("whhT", [H, H], FW, kind="ExternalInput")  # [k, j]
    wihT_d = nc.dram_tensor("wihT", [I, H], F16, kind="ExternalInput")  # [i, j]
    wfcT_d = nc.dram_tensor("wfcT", [H, O], F16, kind="ExternalInput")  # [k, o]
    bh_d = nc.dram_tensor("bh", [H], F32, kind="ExternalInput")     # (bih+bhh)*s
    bfc_d = nc.dram_tensor("bfc", [O], F32, kind="ExternalInput")
    h0_d = nc.dram_tensor("h0r", [128, KJ * BL], F16, kind="ExternalInput")
    nrep_d = (nc.dram_tensor("nrep", [1, 1], mybir.dt.int32,
                             kind="ExternalInput") if dyn_repeat else None)
    out_d = nc.dram_tensor("out", [BL, O, t_total], F32, kind="ExternalOutput")

    with tile.TileContext(nc) as tc_ctx:
        with (
            tc_ctx.tile_pool(name="const", bufs=1) as cpool,
            tc_ctx.tile_pool(name="xT", bufs=2) as xT_pool,
            tc_ctx.tile_pool(name="xp", bufs=2) as xp_pool,
            tc_ctx.tile_pool(name="hs", bufs=2) as hs_pool,
            tc_ctx.tile_pool(name="outT", bufs=2) as outT_pool,
            tc_ctx.tile_pool(name="ppza", bufs=psum_z // 2,
                             space="PSUM") as ppza_pool,
            tc_ctx.tile_pool(name="ppzb", bufs=psum_z // 2,
                             space="PSUM") as ppzb_pool,
            tc_ctx.tile_pool(name="pbig", bufs=8 - psum_z,
                             space="PSUM") as pbig_pool,
        ):
            # ---- constants ----
            ident = cpool.tile([128, 128], FW)
            make_identity(nc, ident[:])

            wT = cpool.tile([128, KJ * H], FW)       # [k-part, (kc, j)]
            nc.sync.dma_start(wT[:].rearrange("p (c j) -> p c j", c=KJ),
                              whhT_d[:].rearrange("(c p) j -> p c j", p=128))
            wih = cpool.tile([128, H], F16)          # [i, j]
            nc.sync.dma_start(wih[:], wihT_d[:])
            wfc = cpool.tile([128, KJ * O], F16)     # [k-part, (kc, o)]
            nc.sync.dma_start(wfc[:].rearrange("p (c o) -> p c o", c=KJ),
                              wfcT_d[:].rearrange("(c p) o -> p c o", p=128))
            bh = cpool.tile([128, KJ], F32)
            nc.sync.dma_start(bh[:], bh_d[:].rearrange("(c p) -> p c", p=128))
            bfc = cpool.tile([128, 1], F32)
            nc.sync.dma_start(bfc[:], bfc_d[:].rearrange("(p o) -> p o", o=1))
            h0sb = cpool.tile([128, KJ * BL], F16)
            nc.sync.dma_start(h0sb[:], h0_d[:])

            def stage1_unit(c, xT):
                """one DMA: the chunk's pre-transposed x [i, (t, b)] fp16"""
                nc.sync.dma_start(xT[:], xT_d[:, ds(c * tc * BL, tc * BL)])

            def stage2_unit(xT, xp, jc, blk):
                """one block of xp = WihT.T @ xT + bias, layout (t, jc, b)"""
                xp_r = xp[:].rearrange("p (t j b) -> p t j b", j=KJ, b=BL)
                pp = pbig_pool.tile([128, 512], F32, tag="big")
                nc.tensor.matmul(pp[:], wih[:, ts(jc, 128)],
                                 xT[:, ds(blk * 512, 512)],
                                 start=True, stop=True)
                nc.vector.tensor_scalar_add(
                    xp_r[:, ds(blk * 64, 64), jc, :],
                    pp[:].rearrange("p (t b) -> p t b", b=BL),
                    bh[:, ds(jc, 1)])

            def stage12_units(c, xT, xp):
                yield lambda: stage1_unit(c, xT)
                for jc in range(KJ):
                    for blk in range(nblk):
                        yield lambda jc=jc, blk=blk: stage2_unit(xT, xp, jc, blk)

            def stage3_recur(c, xp, hs_prev_r, pending=()):
                xp_r = xp[:].rearrange("p (t j b) -> p t j b", j=KJ, b=BL)
                hs = hs_pool.tile([128, KJ * tc * BL], F16)
                hs_r = hs[:].rearrange("p (k t b) -> p k t b", k=KJ, b=BL)

                def rhs_for(t, kc, g):
                    if static_rhs:
                        return h0sb[:, ds(kc * BL + g * GB, GB)]
                    if t > 0:
                        return hs_r[:, kc, t - 1, ds(g * GB, GB)]
                    if c > 0:
                        return hs_prev_r[:, kc, tc - 1, ds(g * GB, GB)]
                    return h0sb[:, ds(kc * BL + g * GB, GB)]

                pools = [ppza_pool, ppzb_pool]

                def inject(t, g):
                    # xp (+biases) injected into a fresh PSUM bank via an
                    # identity matmul; emitted one step AHEAD of its
                    # consumers so the later tanh's semaphore threshold
                    # stays exactly at its real producer matmul.
                    pz = pools[g].tile([128, KJ * GB], F32, tag="pz",
                                       name="pz")
                    nc.tensor.matmul(pz[:], ident[:],
                                     xp_r[:, t, :, ds(g * GB, GB)],
                                     start=True, stop=False,
                                     skip_group_check=True)
                    return pz

                def gstep(t, g, pz):
                    # one group's step: 16 recurrence matmuls (kc-outer)
                    # accumulating onto the pre-injected xp, then one tanh
                    pz_r = pz[:].rearrange("p (j b) -> p j b", b=GB)
                    for kc in range(KJ):
                        for jc in range(KJ):
                            nc.tensor.matmul(
                                pz_r[:, jc, :],
                                wT[:, ds(kc * H + jc * 128, 128)],
                                rhs_for(t, kc, g), start=False,
                                stop=(kc == KJ - 1),
                                skip_group_check=True)
                    if not no_act:
                        sc = 1.0 / WSCALE if w8 else 1.0
                        if split_act:
                            for s in range(2):
                                nc.scalar.activation(
                                    hs_r[:, ds(2 * s, 2), t, ds(g * GB, GB)],
                                    pz_r[:, ds(2 * s, 2), :], AFT.Tanh,
                                    scale=sc)
                        else:
                            nc.scalar.activation(
                                hs_r[:, :, t, ds(g * GB, GB)],
                                pz_r[:, :, :], AFT.Tanh, scale=sc)

                n_pend = len(pending)
                emitted = 0
                nu = tc * NG
                u = 0
                pzs = [inject(0, g) for g in range(NG)]
                for t in range(tc):
                    for g in range(NG):
                        pz = pzs[g]
                        if t + 1 < tc:
                            pzs[g] = inject(t + 1, g)
                        gstep(t, g, pz)
                        u += 1
                        # spread boundary work (prev readout, next load/
                        # xproj) between group bursts, where the PE's
                        # dependency slack lives
                        if (u * n_pend) // nu > emitted:
                            pending[emitted]()
                            emitted += 1
                while emitted < n_pend:
                    pending[emitted]()
                    emitted += 1
                if no_act:
                    nc.vector.memset(hs[:], 0.0)
                return hs, hs_r

            def stage4_unit(hs_r, outT, blk):
                """one readout block: outT[o, (b, t)] = WfcT.T @ hs + b_fc"""
                outT_tb = outT[:].rearrange("p (b t) -> p t b", b=BL)
                po = pbig_pool.tile([128, 512], F32, tag="big")
                for kc in range(KJ):
                    nc.tensor.matmul(
                        po[:], wfc[:, ts(kc, 128)],
                        hs_r[:, kc, ds(blk * 64, 64), :],
                        start=(kc == 0), stop=(kc == KJ - 1))
                nc.vector.tensor_scalar_add(
                    outT_tb[:, ds(blk * 64, 64), :],
                    po[:].rearrange("p (t b) -> p t b", b=BL),
                    bfc[:, 0:1])

            def stage5_unit(c, outT, b):
                """DMA one batch row [o, t] of the chunk out (transposed)"""
                nc.sync.dma_start(out_d[b, :, ds(c * tc, tc)],
                                  outT[:, ds(b * tc, tc)])

            def stage45_units(c, hs_r):
                outT = outT_pool.tile([128, tc * BL], F32)
                for blk in range(nblk):
                    yield lambda blk=blk: stage4_unit(hs_r, outT, blk)
                for b in range(BL):
                    yield lambda b=b: stage5_unit(c, outT, b)

            def emit_all():
                # chunk 0 prologue
                xT = xT_pool.tile([128, tc * BL], F16, tag="xT")
                xp = xp_pool.tile([128, tc * KJ * BL], F16, tag="xp")
                for u in stage12_units(0, xT, xp):
                    u()
                hs_prev_r = None
                for c in range(nt):
                    # work to interleave into this chunk's recurrence:
                    # previous chunk's readout + next chunk's load/xproj
                    pending = []
                    if hs_prev_r is not None:
                        pending.extend(stage45_units(c - 1, hs_prev_r))
                    if c + 1 < nt:
                        xT_n = xT_pool.tile([128, tc * BL], F16, tag="xT")
                        xp_n = xp_pool.tile([128, tc * KJ * BL], F16, tag="xp")
                        pending.extend(stage12_units(c + 1, xT_n, xp_n))
                    else:
                        xT_n = xp_n = None
                    hs, hs_r = stage3_recur(c, xp, hs_prev_r, pending)
                    hs_prev_r = hs_r
                    xT, xp = xT_n, xp_n
                # last chunk epilogue
                for u in stage45_units(nt - 1, hs_prev_r):
                    u()

            if dyn_repeat:
                nrep_sb = cpool.tile([1, 1], mybir.dt.int32)
                nc.sync.dma_start(nrep_sb[:], nrep_d[:])
                rep_val = nc.values_load(nrep_sb[0:1, 0:1], min_val=0,
                                         max_val=64,
                                         skip_runtime_bounds_check=True)
                with tc_ctx.For_i(0, rep_val, 1):
                    emit_all()
            else:
                emit_all()

    nc.compile()
    return nc


def _prep_in_maps(x, h0, W_ih, b_ih, W_hh, b_hh, mask_ih, mask_hh, W_fc, b_fc,
                  t_total=T, wdtype="f8"):
    # With fp8 weights, the whole pre-tanh accumulation (Whh@h and xp
    # alike) is scaled by WSCALE so Whh fits fp8e3m4 well; tanh's scale
    # undoes it.
    w8 = wdtype == "f8"
    s = WSCALE if w8 else 1.0
    import ml_dtypes
    wnp = {"f8": ml_dtypes.float8_e3m4, "f16": np.float16,
           "bf16": ml_dtypes.bfloat16}[wdtype]
    whhT = np.ascontiguousarray(
        (np.asarray(W_hh) * np.asarray(mask_hh)).T * s).astype(wnp)
    wihT = np.ascontiguousarray(
        (np.asarray(W_ih) * np.asarray(mask_ih)).T * s).astype(np.float16)
    wfcT = np.ascontiguousarray(np.asarray(W_fc).T).astype(np.float16)
    bh = ((np.asarray(b_ih) + np.asarray(b_hh)) * s).astype(np.float32)
    bfc = np.asarray(b_fc).astype(np.float32)
    x = np.asarray(x)
    h0 = np.asarray(h0)
    in_maps = []
    for core in range(NCORES):
        bs = core * BL
        h0s = h0[0, bs:bs + BL, :].astype(np.float16)          # [BL, H]
        h0r = np.ascontiguousarray(
            h0s.T.reshape(KJ, 128, BL).transpose(1, 0, 2).reshape(128, KJ * BL))
        # [BL, T, I] -> [I, T, BL] fp16, flattened [128, T*BL]
        xT = np.ascontiguousarray(
            x[bs:bs + BL, :t_total, :].transpose(2, 1, 0)).astype(
                np.float16).reshape(128, t_total * BL)
        in_maps.append({
            "xT": xT,
            "whhT": whhT, "wihT": wihT, "wfcT": wfcT,
            "bh": bh, "bfc": bfc, "h0r": h0r,
        })
    return in_maps


def kernel(x, h0, W_ih, b_ih, W_hh, b_hh, mask_ih, mask_hh, W_fc, b_fc):
    if "nc" not in _cache:
        _cache["nc"] = build_rnn()
    nc = _cache["nc"]
    in_maps = _prep_in_maps(x, h0, W_ih, b_ih, W_hh, b_hh,
                            mask_ih, mask_hh, W_fc, b_fc)
    res = run_bass_kernel_spmd(nc, in_maps, list(range(NCORES)))
    # device output is [BL, O, T]; un-transpose to [BL, T, O]
    return np.concatenate(
        [res.results[c]["out"].transpose(0, 2, 1) for c in range(NCORES)],
        axis=0).astype(np.float32)


# revision 24
# speedup vs baseline: 1.4625x; 1.2212x over previous
"""Trainium2 Bass kernel for the BioRNN problem.

Math (per batch element b):
    Wih_m = W_ih * mask_ih            [H, I]
    Whh_m = W_hh * mask_hh            [H, H]
    xp[t] = Wih_m @ x[t] + b_ih + b_hh
    h[t]  = tanh(xp[t] + Whh_m @ h[t-1])
    out[t] = W_fc @ h[t] + b_fc

Strategy: data-parallel over batch (B=64 -> 8 per NeuronCore), weights
replicated, the T=2048 time scan runs locally per core with no
cross-core communication.

Per-core layout (all matmul operands fp16, fp32 PSUM accumulation):
  - hidden state kept transposed: hT [H on partitions (4 chunks of 128),
    batch(8) on free]. Recurrence matmul is "weights stationary":
        z.T[j,:] += WhhT[k-chunk, j-chunk].T @ hT[k-chunk]
    which keeps the layout stable step to step.
  - xp is precomputed in bulk per time-chunk and injected into the PSUM
    accumulation via an identity matmul (start=True), so the per-step
    chain is just PE(17 matmuls) -> ACT(tanh) -> PE.
  - x is loaded [t, i], PE-transposed to [i, (t, b)] for the bulk xproj.
  - readout is a bulk matmul over each time-chunk, then PE-transposed
    back to [t, (b, o)] for contiguous DMA out.
"""

import numpy as np

import concourse.bacc as bacc
import concourse.mybir as mybir
import concourse.tile as tile
from concourse.bass import ds, ts
from concourse.masks import make_identity
from concourse.bass_utils import run_bass_kernel_spmd

F32 = mybir.dt.float32
F16 = mybir.dt.float16
BF16 = mybir.dt.bfloat16
F8 = mybir.dt.float8e3     # e3m4: 4 mantissa bits, max 15.5
AFT = mybir.ActivationFunctionType

B, T, I, H, O = 64, 2048, 128, 512, 128
NCORES = 8
BL = B // NCORES            # 8 batch elements per core
KJ = H // 128               # 4 hidden chunks
WSCALE = 32.0               # Whh prescale for fp8e3m4; undone in tanh

_cache = {}


def build_rnn(t_total=T, tc=512, act_split="pipe2", static_rhs=False, no_act=False,
              dyn_repeat=False, alt_order=False, psum_bufs=2, wdtype="bf16"):
    """Build (and bacc-compile) the per-core Bass program.

    static_rhs/no_act are timing-diagnostic switches (wrong results):
    static_rhs breaks the cross-step dependency (recurrence always reads
    h0), no_act drops the tanh instructions. dyn_repeat adds an "nrep"
    input and wraps the whole computation in a hardware loop for
    slope-based device timing.
    """
    assert t_total % tc == 0 and tc % 128 == 0
    nt = t_total // tc       # number of time chunks
    ntau = tc // 128         # 128-row t-tiles per chunk per batch
    nblk = tc * BL // 512    # 512-col blocks per chunk

    nc = bacc.Bacc("TRN2", target_bir_lowering=False, debug=False,
                   num_devices=NCORES)

    w8 = wdtype == "f8"
    FW = {"f8": F8, "f16": F16, "bf16": BF16}[wdtype]  # recurrence stationary
    xT_d = nc.dram_tensor("xT", [128, t_total * BL], F16,
                          kind="ExternalInput")                       # [i,(t,b)]
    whhT_d = nc.dram_tensor("whhT", [H, H], FW, kind="ExternalInput")   # [k, j]
    wihT_d = nc.dram_tensor("wihT", [I, H], F16, kind="ExternalInput")   # [i, j]
    wfcT_d = nc.dram_tensor("wfcT", [H, O], F16, kind="ExternalInput")   # [k, o]
    bh_d = nc.dram_tensor("bh", [H], F32, kind="ExternalInput")          # b_ih+b_hh
    bfc_d = nc.dram_tensor("bfc", [O], F32, kind="ExternalInput")
    h0_d = nc.dram_tensor("h0r", [128, KJ * BL], F16, kind="ExternalInput")
    nrep_d = (nc.dram_tensor("nrep", [1, 1], mybir.dt.int32,
                             kind="ExternalInput") if dyn_repeat else None)
    out_d = nc.dram_tensor("out", [BL, O, t_total], F32, kind="ExternalOutput")

    with tile.TileContext(nc) as tc_ctx:
        with (
            tc_ctx.tile_pool(name="const", bufs=1) as cpool,
            tc_ctx.tile_pool(name="xT", bufs=2) as xT_pool,
            tc_ctx.tile_pool(name="xp", bufs=2) as xp_pool,
            tc_ctx.tile_pool(name="hs", bufs=2) as hs_pool,
            tc_ctx.tile_pool(name="outT", bufs=2) as outT_pool,
            tc_ctx.tile_pool(name="ppz", bufs=psum_bufs, space="PSUM") as ppz_pool,
            tc_ctx.tile_pool(name="ppzb", bufs=psum_bufs, space="PSUM") as ppzb_pool,
            tc_ctx.tile_pool(name="pbig", bufs=8 - 2 * psum_bufs,
                             space="PSUM") as pbig_pool,
        ):
            # ---- constants ----
            ident16 = cpool.tile([128, 128], FW)
            make_identity(nc, ident16[:])

            wT = cpool.tile([128, KJ * H], FW)       # [k-part, (kc, j)]
            nc.sync.dma_start(wT[:].rearrange("p (c j) -> p c j", c=KJ),
                              whhT_d[:].rearrange("(c p) j -> p c j", p=128))
            wih = cpool.tile([128, H], F16)          # [i, j]
            nc.sync.dma_start(wih[:], wihT_d[:])
            wfc = cpool.tile([128, KJ * O], F16)     # [k-part, (kc, o)]
            nc.sync.dma_start(wfc[:].rearrange("p (c o) -> p c o", c=KJ),
                              wfcT_d[:].rearrange("(c p) o -> p c o", p=128))
            bh = cpool.tile([128, KJ], F32)
            nc.sync.dma_start(bh[:], bh_d[:].rearrange("(c p) -> p c", p=128))
            bfc = cpool.tile([128, 1], F32)
            nc.sync.dma_start(bfc[:], bfc_d[:].rearrange("(p o) -> p o", o=1))
            h0sb = cpool.tile([128, KJ * BL], F16)
            nc.sync.dma_start(h0sb[:], h0_d[:])

            def stage1_unit(c, xT):
                """one DMA: the chunk's pre-transposed x [i, (t, b)] fp16"""
                nc.sync.dma_start(xT[:], xT_d[:, ds(c * tc * BL, tc * BL)])

            def stage2_unit(xT, xp, jc, blk):
                """one block of xp = WihT.T @ xT + bias, layout (t, jc, b)"""
                xp_r = xp[:].rearrange("p (t j b) -> p t j b", j=KJ, b=BL)
                pp = pbig_pool.tile([128, 512], F32, tag="big")
                nc.tensor.matmul(pp[:], wih[:, ts(jc, 128)],
                                 xT[:, ds(blk * 512, 512)],
                                 start=True, stop=True)
                nc.vector.tensor_scalar_add(
                    xp_r[:, ds(blk * 64, 64), jc, :],
                    pp[:].rearrange("p (t b) -> p t b", b=BL),
                    bh[:, ds(jc, 1)])

            def stage12_units(c, xT, xp):
                yield lambda: stage1_unit(c, xT)
                for jc in range(KJ):
                    for blk in range(nblk):
                        yield lambda jc=jc, blk=blk: stage2_unit(xT, xp, jc, blk)

            def stage3_recur(c, xp, hs_prev_r, pending=()):
                hs = hs_pool.tile([128, KJ * tc * BL], F16)
                hs_r = hs[:].rearrange("p (k t b) -> p k t b", k=KJ, b=BL)

                def rhs_for(t, kc):
                    if static_rhs:
                        return h0sb[:, ts(kc, BL)]
                    if t > 0:
                        return hs_r[:, kc, t - 1, :]
                    if c > 0:
                        return hs_prev_r[:, kc, tc - 1, :]
                    return h0sb[:, ts(kc, BL)]

                def step_pipe2(t, ve_add=False, filler=None):
                    # Two psum banks (same bank would serialize: PE-write +
                    # ACT-read of one bank is a fatal collision). The tanh
                    # production order ALTERNATES each step so the critical
                    # dependency cycle (last-produced hs half -> its consumer
                    # matmuls -> its next tanh) contains only ONE tanh
                    # instruction, not both serialized on the scalar engine.
                    # Matmul phases consume hs halves in the order step t-1
                    # produced them.
                    pza = ppz_pool.tile([128, 2 * BL], F32, tag="pza")
                    pzb = ppzb_pool.tile([128, 2 * BL], F32, tag="pzb")
                    pzs = [pza, pzb]
                    if alt_order:
                        prod = (0, 1) if t % 2 == 0 else (1, 0)
                        cons = (0, 1) if (t - 1) % 2 == 0 else (1, 0)
                    else:
                        prod = (0, 1)
                        cons = (0, 1)
                    for h_i in prod:
                        nc.tensor.matmul(
                            pzs[h_i][:], ident16[:],
                            xp[:, ds(t * KJ * BL + h_i * 2 * BL, 2 * BL)],
                            start=True, stop=False, skip_group_check=True)
                    for ph, ch in enumerate(cons):
                        if ph == 1 and filler is not None:
                            # foreign PE work placed in the window where the
                            # PE would stall waiting for the late tanh half
                            # of step t-1
                            filler()
                        for h_i in prod:
                            pz = pzs[h_i]
                            for jc in (2 * h_i, 2 * h_i + 1):
                                for kc in (2 * ch, 2 * ch + 1):
                                    nc.tensor.matmul(
                                        pz[:, ts(jc - 2 * h_i, BL)],
                                        wT[:, ds(kc * H + jc * 128, 128)],
                                        rhs_for(t, kc),
                                        start=False,
                                        stop=(ph == 1 and kc == 2 * ch + 1),
                                        skip_group_check=True)
                            if ph == 1 and not no_act:
                                nc.scalar.activation(
                                    hs_r[:, ds(2 * h_i, 2), t, :],
                                    pz[:].rearrange("p (j b) -> p j b", b=BL),
                                    AFT.Tanh,
                                    scale=1.0 / WSCALE if w8 else 1.0)

                def step_plain(t):
                    pz = ppz_pool.tile([128, KJ * BL], F32)
                    pz_r = pz[:].rearrange("p (j b) -> p j b", b=BL)
                    # inject xp (+biases) into the accumulator
                    nc.tensor.matmul(pz[:], ident16[:], xp[:, ts(t, KJ * BL)],
                                     start=True, stop=False,
                                     skip_group_check=True)
                    for jc in range(KJ):
                        for kc in range(KJ):
                            nc.tensor.matmul(
                                pz_r[:, jc, :],
                                wT[:, ds(kc * H + jc * 128, 128)],
                                rhs_for(t, kc), start=False,
                                stop=(kc == KJ - 1),
                                skip_group_check=True)
                    if not no_act:
                        span = KJ // act_split
                        for s in range(act_split):
                            nc.scalar.activation(
                                hs_r[:, ds(s * span, span), t, :],
                                pz_r[:, ds(s * span, span), :], AFT.Tanh,
                                scale=1.0 / WSCALE if w8 else 1.0)

                n_pend = len(pending)
                emitted = 0
                for t in range(tc):
                    # spread boundary work (prev readout, next load/xproj)
                    # into the recurrence, where PE has idle slots
                    filler = None
                    if (t + 1) * n_pend // tc > emitted:
                        unit = pending[emitted]
                        emitted += 1
                        filler = unit
                    if act_split == "pipe2":
                        step_pipe2(t, filler=filler)
                    elif act_split == "pipe2v":
                        step_pipe2(t, ve_add=True, filler=filler)
                    else:
                        step_plain(t)
                        if filler is not None:
                            filler()
                if no_act:
                    # keep hs defined for the readout stage
                    nc.vector.memset(hs[:], 0.0)
                return hs, hs_r

            def stage4_unit(hs_r, outT, blk):
                """one readout block: outT[o, (b-major t)] = WfcT.T@hs + b_fc"""
                outT_tb = outT[:].rearrange("p (b t) -> p t b", b=BL)
                po = pbig_pool.tile([128, 512], F32, tag="big")
                for kc in range(KJ):
                    nc.tensor.matmul(
                        po[:], wfc[:, ts(kc, 128)],
                        hs_r[:, kc, ds(blk * 64, 64), :],
                        start=(kc == 0), stop=(kc == KJ - 1))
                nc.vector.tensor_scalar_add(
                    outT_tb[:, ds(blk * 64, 64), :],
                    po[:].rearrange("p (t b) -> p t b", b=BL),
                    bfc[:, 0:1])

            def stage5_unit(c, outT, b):
                """DMA one batch row [o, t] of the chunk out (transposed)"""
                nc.sync.dma_start(out_d[b, :, ds(c * tc, tc)],
                                  outT[:, ds(b * tc, tc)])

            def stage45_units(c, hs_r):
                outT = outT_pool.tile([128, tc * BL], F32)
                for blk in range(nblk):
                    yield lambda blk=blk: stage4_unit(hs_r, outT, blk)
                for b in range(BL):
                    yield lambda b=b: stage5_unit(c, outT, b)

            def emit_all():
                # chunk 0 prologue
                xT = xT_pool.tile([128, tc * BL], F16, tag="xT")
                xp = xp_pool.tile([128, tc * KJ * BL], F16, tag="xp")
                for u in stage12_units(0, xT, xp):
                    u()
                hs_prev_r = None
                for c in range(nt):
                    # work to interleave into this chunk's recurrence:
                    # previous chunk's readout + next chunk's load/xproj
                    pending = []
                    if hs_prev_r is not None:
                        pending.extend(stage45_units(c - 1, hs_prev_r))
                    if c + 1 < nt:
                        xT_n = xT_pool.tile([128, tc * BL], F16, tag="xT")
                        xp_n = xp_pool.tile([128, tc * KJ * BL], F16, tag="xp")
                        pending.extend(stage12_units(c + 1, xT_n, xp_n))
                    else:
                        xT_n = xp_n = None
                    hs, hs_r = stage3_recur(c, xp, hs_prev_r, pending)
                    hs_prev_r = hs_r
                    xT, xp = xT_n, xp_n
                # last chunk epilogue
                for u in stage45_units(nt - 1, hs_prev_r):
                    u()

            if dyn_repeat:
                nrep_sb = cpool.tile([1, 1], mybir.dt.int32)
                nc.sync.dma_start(nrep_sb[:], nrep_d[:])
                rep_val = nc.values_load(nrep_sb[0:1, 0:1], min_val=0,
                                         max_val=64,
                                         skip_runtime_bounds_check=True)
                with tc_ctx.For_i(0, rep_val, 1):
                    emit_all()
            else:
                emit_all()

    nc.compile()
    return nc


def _prep_in_maps(x, h0, W_ih, b_ih, W_hh, b_hh, mask_ih, mask_hh, W_fc, b_fc,
                  t_total=T, wdtype="bf16"):
    # With fp8 weights, the whole pre-tanh accumulation (Whh@h and xp
    # alike) is scaled by WSCALE so Whh fits fp8e3m4 well; tanh's scale
    # undoes it.
    import ml_dtypes
    w8 = wdtype == "f8"
    s = WSCALE if w8 else 1.0
    wnp = {"f8": ml_dtypes.float8_e3m4, "f16": np.float16,
           "bf16": ml_dtypes.bfloat16}[wdtype]
    whhT = np.ascontiguousarray(
        (np.asarray(W_hh) * np.asarray(mask_hh)).T * s).astype(wnp)
    wihT = np.ascontiguousarray(
        (np.asarray(W_ih) * np.asarray(mask_ih)).T * s).astype(np.float16)
    wfcT = np.ascontiguousarray(np.asarray(W_fc).T).astype(np.float16)
    bh = ((np.asarray(b_ih) + np.asarray(b_hh)) * s).astype(np.float32)
    bfc = np.asarray(b_fc).astype(np.float32)
    x = np.asarray(x)
    h0 = np.asarray(h0)
    in_maps = []
    for core in range(NCORES):
        bs = core * BL
        h0s = h0[0, bs:bs + BL, :].astype(np.float16)          # [BL, H]
        h0r = np.ascontiguousarray(
            h0s.T.reshape(KJ, 128, BL).transpose(1, 0, 2).reshape(128, KJ * BL))
        # [BL, T, I] -> [I, T, BL] fp16, flattened [128, T*BL]
        xT = np.ascontiguousarray(
            x[bs:bs + BL, :t_total, :].transpose(2, 1, 0)).astype(
                np.float16).reshape(128, t_total * BL)
        in_maps.append({
            "xT": xT,
            "whhT": whhT, "wihT": wihT, "wfcT": wfcT,
            "bh": bh, "bfc": bfc, "h0r": h0r,
        })
    return in_maps


def kernel(x, h0, W_ih, b_ih, W_hh, b_hh, mask_ih, mask_hh, W_fc, b_fc):
    if "nc" not in _cache:
        _cache["nc"] = build_rnn()
    nc = _cache["nc"]
    in_maps = _prep_in_maps(x, h0, W_ih, b_ih, W_hh, b_hh,
                            mask_ih, mask_hh, W_fc, b_fc)
    res = run_bass_kernel_spmd(nc, in_maps, list(range(NCORES)))
    # device output is [BL, O, T]; un-transpose to [BL, T, O]
    return np.concatenate(
        [res.results[c]["out"].transpose(0, 2, 1) for c in range(NCORES)],
        axis=0).astype(np.float32)

